# revision 2
# baseline (speedup 1.0000x reference)
"""Variable-length average pooling (prefix mean over seq axis) on 8 trn2 cores.

Strategy (dense row packing + all-PE bf16 reduction):
  - eff_len[b] = lengths[b] if >0 else L.  pooled[b] = sum_{l<eff} x[b,l,:] / eff.
  - Batches are snake-assigned by sorted eff so per-core total rows balance to
    ~0.6%.  The HOST then packs each core's valid rows densely into one
    contiguous [N_CH*128, 2048] fp32 buffer (invalid rows never uploaded), so
    the device reads exactly ceil(total_rows/128) full 1 MiB chunks -- ~68 MiB
    per core vs 78 MiB for the per-slot-padded layout.
  - Chunks are fetched in 2 MiB pairs alternating the two HWDGE rings
    (SP/ACT); each ring sustains ~200 GB/s, so balanced rings give ~400 GB/s.
    Optional third SWDGE ring (gpsimd) via DMA_QUEUES=3.
  - fp32 moving operands run the PE at 1/4 rate, so each pair is first cast
    fp32->bf16 by the DVE (2x_2P mode, ~2.2us per 2 MiB pair; rel-err of the
    bf16 mean ~1e-3 << the 2e-2 gate).  The PE then reduces every chunk with
    one accumulation group: stationary mask[128 rows, 16 batch-cols] (weight
    1/eff on rows belonging to that batch, 0 elsewhere -- a chunk spanning
    several batches just has several nonzero columns), moving bf16 chunk
    [128, 512], psum[16, 512] x4 banks.  Rows at batch boundaries need no
    alignment because masking, not slicing, selects them.
  - End: one DVE copy psum[16,2048]->SBUF, one 128 KiB DMA out.
"""

import os

import numpy as np
import ml_dtypes

import concourse.bacc as bacc
import concourse.mybir as mybir
from concourse.tile import TileContext
from concourse.bass_utils import run_bass_kernel_spmd

B, L, D = 128, 1024, 2048
NCORES = 8
SLOTS = B // NCORES  # 16 batches per core
PCHUNK = 128         # rows per chunk (partition dim)
NTILE = 512          # matmul moving free dim (one PSUM bank of fp32)

PAIR_BUFS = int(os.environ.get("PAIR_BUFS", "6"))
BF16_BUFS = int(os.environ.get("BF16_BUFS", "3"))
DMA_QUEUES = int(os.environ.get("DMA_QUEUES", "2"))

LAST_RESULTS = None  # BassKernelResults of the most recent device run


def _assign(eff):
    """Snake-assign sorted batches to cores to balance total rows."""
    order = np.argsort(-eff, kind="stable")
    cores = [[] for _ in range(NCORES)]
    for i, idx in enumerate(order):
        blk, pos = divmod(i, NCORES)
        c = pos if blk % 2 == 0 else NCORES - 1 - pos
        cores[c].append(int(idx))
    return cores


_PROGRAM_CACHE = {}


def _build_program(n_ch):
    nc = bacc.Bacc(None, target_bir_lowering=False)
    f32 = mybir.dt.float32
    bf16 = mybir.dt.bfloat16
    rows = n_ch * PCHUNK
    feat = nc.dram_tensor("features", [rows, D], f32, kind="ExternalInput")
    maskt = nc.dram_tensor("maskt", [PCHUNK, n_ch * SLOTS], bf16, kind="ExternalInput")
    out = nc.dram_tensor("out", [SLOTS, D], f32, kind="ExternalOutput")

    with TileContext(nc) as tc:
        with (
            tc.tile_pool(name="mask", bufs=1) as mpool,
            tc.tile_pool(name="pairs", bufs=PAIR_BUFS) as fpool,
            tc.tile_pool(name="halfs", bufs=BF16_BUFS) as hpool,
            tc.tile_pool(name="psum", bufs=1, space="PSUM") as ppool,
            tc.tile_pool(name="outs", bufs=1) as opool,
        ):
            mask_tile = mpool.tile([PCHUNK, n_ch * SLOTS], bf16)
            nc.sync.dma_start(out=mask_tile[:], in_=maskt[:])
            dma_engines = [nc.scalar, nc.sync, nc.gpsimd][:DMA_QUEUES]

            psum = ppool.tile([SLOTS, D], f32)

            def compute(tile_bf16, off, t):
                for j in range(D // NTILE):
                    nc.tensor.matmul(
                        psum[:, j * NTILE : (j + 1) * NTILE],
                        mask_tile[:, t * SLOTS : (t + 1) * SLOTS],
                        tile_bf16[:, off + j * NTILE : off + (j + 1) * NTILE],
                        start=(t == 0),
                        stop=(t == n_ch - 1),
                    )

            n_dma = 0
            t = 0
            while t < n_ch:
                if t + 1 < n_ch:
                    pair = fpool.tile([PCHUNK, 2 * D], f32, name="pair", tag="p")
                    src = feat[t * PCHUNK : (t + 2) * PCHUNK, :].rearrange(
                        "(c p) d -> p c d", p=PCHUNK
                    )
                    dst = pair[:].rearrange("p (c d) -> p c d", c=2)
                    dma_engines[n_dma % DMA_QUEUES].dma_start(out=dst, in_=src)
                    bpair = hpool.tile([PCHUNK, 2 * D], bf16, name="bpair", tag="h")
                    nc.vector.tensor_copy(out=bpair[:], in_=pair[:])
                    compute(bpair, 0, t)
                    compute(bpair, D, t + 1)
                    t += 2
                else:
                    single = fpool.tile([PCHUNK, D], f32, name="single", tag="p")
                    dma_engines[n_dma % DMA_QUEUES].dma_start(
                        out=single[:], in_=feat[t * PCHUNK : (t + 1) * PCHUNK, :]
                    )
                    bsing = hpool.tile([PCHUNK, D], bf16, name="bsing", tag="h")
                    nc.vector.tensor_copy(out=bsing[:], in_=single[:])
                    compute(bsing, 0, t)
                    t += 1
                n_dma += 1

            out_t = opool.tile([SLOTS, D], f32)
            nc.vector.tensor_copy(out=out_t[:], in_=psum[:])
            nc.sync.dma_start(out=out[:], in_=out_t[:])
    nc.finalize()
    return nc


def kernel(features, lengths):
    global LAST_RESULTS
    features = np.ascontiguousarray(features, dtype=np.float32)
    lengths = np.ascontiguousarray(lengths, dtype=np.int32)
    eff = np.where(lengths > 0, lengths, L).astype(np.int64)

    cores = _assign(eff)
    n_ch = max(
        -(-sum(int(eff[b]) for b in cores[c]) // PCHUNK) for c in range(NCORES)
    )
    rows = n_ch * PCHUNK

    key = (n_ch, PAIR_BUFS, BF16_BUFS, DMA_QUEUES)
    if key not in _PROGRAM_CACHE:
        _PROGRAM_CACHE[key] = _build_program(n_ch)
    nc = _PROGRAM_CACHE[key]

    in_maps = []
    for c in range(NCORES):
        perm = cores[c]
        effs = np.array([int(eff[b]) for b in perm], dtype=np.int64)
        cum = np.concatenate([[0], np.cumsum(effs)])
        dense = np.zeros((rows, D), dtype=np.float32)
        for s, b in enumerate(perm):
            dense[cum[s] : cum[s + 1]] = features[b, : effs[s]]
        # mask M[i, s] = 1/eff_s if dense row i belongs to batch-slot s
        row_batch = np.searchsorted(cum[1:], np.arange(rows), side="right")
        valid = row_batch < SLOTS
        M = np.zeros((rows, SLOTS), dtype=np.float32)
        idx = np.nonzero(valid)[0]
        M[idx, row_batch[idx]] = 1.0 / effs[row_batch[idx]]
        maskt = (
            M.reshape(n_ch, PCHUNK, SLOTS)
            .transpose(1, 0, 2)
            .reshape(PCHUNK, n_ch * SLOTS)
            .astype(ml_dtypes.bfloat16)
        )
        in_maps.append({"features": dense, "maskt": maskt})

    trace = os.environ.get("KERNEL_TRACE", "0") == "1"
    LAST_RESULTS = run_bass_kernel_spmd(
        nc,
        in_maps,
        core_ids=list(range(NCORES)),
        trace=trace,
        trace_cores=[0] if trace else None,
    )

    out = np.empty((B, D), dtype=np.float32)
    for c in range(NCORES):
        out[np.asarray(cores[c])] = LAST_RESULTS.results[c]["out"]
    return out


# revision 8
# speedup vs baseline: 1.1005x; 1.1005x over previous
"""Variable-length average pooling (prefix mean over seq axis) on 8 trn2 cores.

Strategy (dense row packing + all-PE bf16 reduction):
  - eff_len[b] = lengths[b] if >0 else L.  pooled[b] = sum_{l<eff} x[b,l,:] / eff.
  - Batches are snake-assigned by sorted eff so per-core total rows balance to
    ~0.6%.  The HOST then packs each core's valid rows densely into one
    contiguous [N_CH*128, 2048] fp32 buffer (invalid rows never uploaded), so
    the device reads exactly ceil(total_rows/128) full 1 MiB chunks -- ~68 MiB
    per core vs 78 MiB for the per-slot-padded layout.
  - Chunks are fetched in 2 MiB pairs alternating the two HWDGE rings
    (SP/ACT); each ring sustains ~200 GB/s, so balanced rings give ~400 GB/s.
    Optional third SWDGE ring (gpsimd) via DMA_QUEUES=3.
  - fp32 moving operands run the PE at 1/4 rate, so each pair is first cast
    fp32->bf16 by the DVE (2x_2P mode, ~2.2us per 2 MiB pair; rel-err of the
    bf16 mean ~1e-3 << the 2e-2 gate).  The PE then reduces every chunk with
    one accumulation group: stationary mask[128 rows, 16 batch-cols] (weight
    1/eff on rows belonging to that batch, 0 elsewhere -- a chunk spanning
    several batches just has several nonzero columns), moving bf16 chunk
    [128, 512], psum[16, 512] x4 banks.  Rows at batch boundaries need no
    alignment because masking, not slicing, selects them.
  - End: one DVE copy psum[16,2048]->SBUF, one 128 KiB DMA out.
"""

import os

import numpy as np
import ml_dtypes

import concourse.bacc as bacc
import concourse.mybir as mybir
from concourse.tile import TileContext
from concourse.bass_utils import run_bass_kernel_spmd

B, L, D = 128, 1024, 2048
NCORES = 8
SLOTS = B // NCORES  # 16 batches per core
PCHUNK = 128         # rows per chunk (partition dim)
NTILE = 512          # matmul moving free dim (one PSUM bank of fp32)

PAIR_BUFS = int(os.environ.get("PAIR_BUFS", "6"))
BF16_BUFS = int(os.environ.get("BF16_BUFS", "3"))
DMA_QUEUES = int(os.environ.get("DMA_QUEUES", "2"))
# If >0, every GPS_EVERY-th load goes on the gpsimd (SWDGE) queue; the rest
# alternate the two HWDGE rings.  0 disables the third queue.
GPS_EVERY = int(os.environ.get("GPS_EVERY", "0"))

LAST_RESULTS = None  # BassKernelResults of the most recent device run


def _assign(eff):
    """Snake-assign sorted batches to cores to balance total rows."""
    order = np.argsort(-eff, kind="stable")
    cores = [[] for _ in range(NCORES)]
    for i, idx in enumerate(order):
        blk, pos = divmod(i, NCORES)
        c = pos if blk % 2 == 0 else NCORES - 1 - pos
        cores[c].append(int(idx))
    return cores


_PROGRAM_CACHE = {}


def _build_program(n_ch):
    nc = bacc.Bacc(None, target_bir_lowering=False)
    f32 = mybir.dt.float32
    bf16 = mybir.dt.bfloat16
    rows = n_ch * PCHUNK
    feat = nc.dram_tensor("features", [rows, D], f32, kind="ExternalInput")
    maskt = nc.dram_tensor("maskt", [PCHUNK, n_ch * SLOTS], bf16, kind="ExternalInput")
    out = nc.dram_tensor("out", [SLOTS, D], f32, kind="ExternalOutput")

    with TileContext(nc) as tc:
        with (
            tc.tile_pool(name="mask", bufs=1) as mpool,
            tc.tile_pool(name="pairs", bufs=PAIR_BUFS) as fpool,
            tc.tile_pool(name="halfs", bufs=BF16_BUFS) as hpool,
            tc.tile_pool(name="psum", bufs=1, space="PSUM") as ppool,
            tc.tile_pool(name="outs", bufs=1) as opool,
        ):
            mask_tile = mpool.tile([PCHUNK, n_ch * SLOTS], bf16)
            # gpsimd (SWDGE) carries the small mask load so neither HWDGE
            # ring is delayed at the start; both rings then carry exactly
            # half the pair loads.
            nc.gpsimd.dma_start(out=mask_tile[:], in_=maskt[:])
            _sel = {"hw": 0, "n": 0}

            def next_engine():
                n = _sel["n"]
                _sel["n"] += 1
                if GPS_EVERY and n % GPS_EVERY == GPS_EVERY - 1:
                    return nc.gpsimd
                if DMA_QUEUES == 3 and not GPS_EVERY:
                    return [nc.scalar, nc.sync, nc.gpsimd][n % 3]
                e = [nc.scalar, nc.sync][_sel["hw"] % 2]
                _sel["hw"] += 1
                return e

            psum = ppool.tile([SLOTS, D], f32)

            def compute(tile_bf16, off, t):
                for j in range(D // NTILE):
                    nc.tensor.matmul(
                        psum[:, j * NTILE : (j + 1) * NTILE],
                        mask_tile[:, t * SLOTS : (t + 1) * SLOTS],
                        tile_bf16[:, off + j * NTILE : off + (j + 1) * NTILE],
                        start=(t == 0),
                        stop=(t == n_ch - 1),
                    )

            # Last 2 chunks go as 1 MiB singles (one per ring) so the final
            # data lands ~2.5us earlier, shortening the serial tail
            # (cast -> matmuls -> psum copy -> out DMA).
            n_single = 2 if n_ch >= 2 and n_ch % 2 == 0 else n_ch % 2
            n_dma = 0
            t = 0
            while t < n_ch - n_single:
                pair = fpool.tile([PCHUNK, 2 * D], f32, name="pair", tag="p")
                src = feat[t * PCHUNK : (t + 2) * PCHUNK, :].rearrange(
                    "(c p) d -> p c d", p=PCHUNK
                )
                dst = pair[:].rearrange("p (c d) -> p c d", c=2)
                next_engine().dma_start(out=dst, in_=src)
                bpair = hpool.tile([PCHUNK, 2 * D], bf16, name="bpair", tag="h")
                nc.vector.tensor_copy(out=bpair[:], in_=pair[:])
                compute(bpair, 0, t)
                compute(bpair, D, t + 1)
                t += 2
                n_dma += 1
            while t < n_ch:
                single = fpool.tile([PCHUNK, D], f32, name="single", tag="p")
                next_engine().dma_start(
                    out=single[:], in_=feat[t * PCHUNK : (t + 1) * PCHUNK, :]
                )
                bsing = hpool.tile([PCHUNK, D], bf16, name="bsing", tag="h")
                nc.vector.tensor_copy(out=bsing[:], in_=single[:])
                compute(bsing, 0, t)
                t += 1
                n_dma += 1

            # Split the psum drain across DVE+ACT and both rings to overlap.
            out_t = opool.tile([SLOTS, D], f32)
            nc.vector.tensor_copy(out=out_t[:, 0 : D // 2], in_=psum[:, 0 : D // 2])
            nc.scalar.copy(out=out_t[:, D // 2 : D], in_=psum[:, D // 2 : D])
            nc.sync.dma_start(out=out[:, 0 : D // 2], in_=out_t[:, 0 : D // 2])
            nc.scalar.dma_start(out=out[:, D // 2 : D], in_=out_t[:, D // 2 : D])
    nc.finalize()
    return nc


def kernel(features, lengths):
    global LAST_RESULTS
    features = np.ascontiguousarray(features, dtype=np.float32)
    lengths = np.ascontiguousarray(lengths, dtype=np.int32)
    eff = np.where(lengths > 0, lengths, L).astype(np.int64)

    cores = _assign(eff)
    n_ch = max(
        -(-sum(int(eff[b]) for b in cores[c]) // PCHUNK) for c in range(NCORES)
    )
    rows = n_ch * PCHUNK

    key = (n_ch, PAIR_BUFS, BF16_BUFS, DMA_QUEUES, GPS_EVERY)
    if key not in _PROGRAM_CACHE:
        _PROGRAM_CACHE[key] = _build_program(n_ch)
    nc = _PROGRAM_CACHE[key]

    in_maps = []
    for c in range(NCORES):
        perm = cores[c]
        effs = np.array([int(eff[b]) for b in perm], dtype=np.int64)
        cum = np.concatenate([[0], np.cumsum(effs)])
        dense = np.zeros((rows, D), dtype=np.float32)
        for s, b in enumerate(perm):
            dense[cum[s] : cum[s + 1]] = features[b, : effs[s]]
        # mask M[i, s] = 1/eff_s if dense row i belongs to batch-slot s
        row_batch = np.searchsorted(cum[1:], np.arange(rows), side="right")
        valid = row_batch < SLOTS
        M = np.zeros((rows, SLOTS), dtype=np.float32)
        idx = np.nonzero(valid)[0]
        M[idx, row_batch[idx]] = 1.0 / effs[row_batch[idx]]
        maskt = (
            M.reshape(n_ch, PCHUNK, SLOTS)
            .transpose(1, 0, 2)
            .reshape(PCHUNK, n_ch * SLOTS)
            .astype(ml_dtypes.bfloat16)
        )
        in_maps.append({"features": dense, "maskt": maskt})

    trace = os.environ.get("KERNEL_TRACE", "0") == "1"
    LAST_RESULTS = run_bass_kernel_spmd(
        nc,
        in_maps,
        core_ids=list(range(NCORES)),
        trace=trace,
        trace_cores=[0] if trace else None,
    )

    out = np.empty((B, D), dtype=np.float32)
    for c in range(NCORES):
        out[np.asarray(cores[c])] = LAST_RESULTS.results[c]["out"]
    return out
